# revision 1
# baseline (speedup 1.0000x reference)
"""Dense 2-layer GAT (4 heads) on 8 Trainium2 NeuronCores.

Distribution: 1D row-parallel over destination nodes. Core c owns rows
R_c = [512c, 512c+512). Each core computes its rows of both GAT layers;
a mid-kernel AllGather exchanges the layer-2 projections [Wh2 | d2]
(fp16, 1.1 MB) instead of h1 (the only cross-core dependency).

On-device layout: attention is built TRANSPOSED, att[j, i] (source node j
on partitions, my rows i on free), so
  - hT[o, i] = sum_j Wh[j, o] * att[j, i] needs no transpose of att,
  - softmax denominators come free as an extra ones-column in lhsT,
  - the layer output hT is exactly the lhsT the next layer's projection
    needs.
x is pre-transposed and cast to fp16 on host (layout/dtype-only prep);
adj is pre-transposed, column-sliced per core, and cast to fp16 ({0,1}
exact). Output is produced as h2T [128, 512] per core and un-transposed
on host.

Everything on the N^2 path is fp16: matmuls run at 1 cycle/row (fp32
is 4), DVE tensor_scalar hits 4x mode and tensor_tensor 2x. The
adjacency tile (4 MB fp16) is loaded ONCE and stays resident in SBUF
for both layers. Layer-1 exp uses a global -2 bias (softmax-exact
shift) so exp(z) stays well inside fp16 range. a_dst is folded into
the weights on host (wt1/wt2), so per-node d values fall out of the
projection matmuls directly.

The logits pipeline is balanced across ScalarE and DVE per head
(dve_prelu_heads=2): heads 0-1 build z on DVE and compute
prelu = max(z, 0.2z) with a DVE mult+max; heads 2-3 fuse the z-build
into ScalarE's Prelu via its per-partition bias port. Exp always runs
on ScalarE, the {0,1} mask multiply on DVE, and 8 accumulating fp16
matmuls per superblock produce hT (+denominator row) in psum. The
layer-1 projection front is interleaved superblock-by-superblock with
attention so ScalarE starts ~15us into the kernel, tails are emitted
inline per head, and the layer-2 gather phase (Wh2|d2 via one fused
matmul block) feeds a fp16 AllGather overlapped with s_rep2 compute.
"""
import sys

if "/opt/trn_rl_repo" not in sys.path:
    sys.path.insert(0, "/opt/trn_rl_repo")

import numpy as np

import concourse.bacc as bacc
import concourse.mybir as mybir
import concourse.tile as tile
from concourse.bass_utils import run_bass_kernel_spmd

F32 = mybir.dt.float32
F16 = mybir.dt.float16
AF = mybir.ActivationFunctionType
OP = mybir.AluOpType

N = 4096
NFEAT = 512
NHID = 256
NEMBED = 128
NHEADS = 4
O1 = 64
O2 = 32
NCORES = 8
R = N // NCORES          # 512 rows per core
ALPHA = 0.2
NT = N // 128            # 32 j-tiles
SB = 8                   # j-tiles per superblock
NSB = NT // SB           # 4 superblocks
BLK1 = O1 + 1            # 65: [Wh_h | ones]
BLK2 = O2 + 1            # 33
GCOLS = NEMBED + NHEADS  # 132: [Wh2 (128) | d2 (4)]
EXP_SHIFT = {1: -2.0, 2: 0.0}   # softmax-invariant bias on the exp


def _build(debug=False, repeat=1):
    nc = bacc.Bacc("TRN2", target_bir_lowering=False, debug=False,
                   num_devices=NCORES)

    xT = nc.dram_tensor("xT", [NFEAT, N], F16, kind="ExternalInput").ap()
    xmT = nc.dram_tensor("xmT", [NFEAT, R], F16, kind="ExternalInput").ap()
    adjT = nc.dram_tensor("adjT", [N, R], F16, kind="ExternalInput").ap()
    W1 = nc.dram_tensor("W1", [NHEADS, NFEAT, O1], F32, kind="ExternalInput").ap()
    wt1 = nc.dram_tensor("wt1", [NFEAT, NHEADS], F32, kind="ExternalInput").ap()
    wt2 = nc.dram_tensor("wt2", [NHID, NHEADS], F32, kind="ExternalInput").ap()
    a1f = nc.dram_tensor("a1f", [1, 2 * O1 * NHEADS], F32, kind="ExternalInput").ap()
    W2 = nc.dram_tensor("W2", [NHEADS, NHID, O2], F32, kind="ExternalInput").ap()
    a2f = nc.dram_tensor("a2f", [1, 2 * O2 * NHEADS], F32, kind="ExternalInput").ap()
    out = nc.dram_tensor("h2T", [NEMBED, R], F32, kind="ExternalOutput").ap()
    dbg = None
    if debug:
        dbg = {
            "H1T": nc.dram_tensor("d_H1T", [128, 2, R], F16,
                                  kind="ExternalOutput").ap(),
            "srep2": nc.dram_tensor("d_srep2", [128, R], F16,
                                    kind="ExternalOutput").ap(),
            "WhD2": nc.dram_tensor("d_WhD2", [128, NT, 4 * BLK2], F16,
                                   kind="ExternalOutput").ap(),
            "D2": nc.dram_tensor("d_D2", [128, NT, NHEADS], F32,
                                 kind="ExternalOutput").ap(),
            "Gsb": nc.dram_tensor("d_Gsb", [128, 4, GCOLS], F16,
                                  kind="ExternalOutput").ap(),
        }

    with tile.TileContext(nc) as tc:
        for _rep in range(repeat):
            _emit(tc, nc, xT, xmT, adjT, W1, wt1, wt2, a1f, W2, a2f, out, dbg)
    nc.compile()
    return nc


def _emit(tc, nc, xT, xmT, adjT, W1, wt1, wt2, a1f, W2, a2f, out, dbg):
    v_ = nc.vector
    s_ = nc.scalar
    t_ = nc.tensor

    with (
        tc.tile_pool(name="persist", bufs=1) as P,
        tc.tile_pool(name="small", bufs=VARIANT.get("sp_bufs", 2)) as SP,
        tc.tile_pool(name="psA", bufs=1, space="PSUM") as PSA,
        tc.tile_pool(name="psB", bufs=VARIANT.get("psb_bufs", 4), space="PSUM") as PSB,
        tc.tile_pool(name="dram", bufs=1, space="DRAM") as DP,
    ):
        XM = P.tile([128, 4, R], F16, tag="XM")
        nc.sync.dma_start(XM[:], xmT[:, :].rearrange("(c p) n -> p c n", p=128))
        ones32 = P.tile([1, 128], F32, tag="ones32")
        v_.memset(ones32[:], 1.0)
        ones16 = P.tile([1, 128], F16, tag="ones16")
        v_.memset(ones16[:], 1.0)

        # ---- resident adjacency: loaded per-superblock inside the L1
        # loop (behind each xq tile on the sync queue), reused by layer 2
        AT = P.tile([128, NT, R], F16, tag="AT")

        # ---- per-head a_src prep for both layers ----------------------
        # One contiguous DMA per layer; broadcasts built via K=1 matmuls.
        asrc_rep = {}   # (l, h) -> [Fo, 128] fp16 a_src broadcast along free
        afsb = {}
        for l, af, Fo in ((1, a1f, O1), (2, a2f, O2)):
            asb = P.tile([1, 2 * Fo * NHEADS], F32, tag=f"afsb{l}")
            nc.sync.dma_start(asb[:], af[:, :])
            afsb[l] = (asb, Fo)
        def _asrc_prep(l):
            asb, Fo = afsb[l]
            for h in range(NHEADS):
                aps = PSB.tile([Fo, 128], F32, tag="ps")
                t_.matmul(aps[:], asb[0:1, 2 * Fo * h: 2 * Fo * h + Fo],
                          ones32[:], start=True, stop=True)
                rep = P.tile([Fo, 128], F16, tag=f"asrc_rep{l}_{h}")
                v_.tensor_copy(rep[:], aps[:])
                asrc_rep[(l, h)] = rep
        _asrc_prep(1)

        VP_ctx = tc.tile_pool(name="vwork", bufs=VARIANT.get("v_bufs", 4))
        VP = VP_ctx.__enter__()
        XP_ctx = tc.tile_pool(name="xload", bufs=1)
        XP = XP_ctx.__enter__()
        # ---- layer-1 front: WR1 = [W1 all heads | w_tilde] fp32, then -
        # fp16 copy WR1h for matmuls; WhD1[:, nt, :] = [Wh_h | 1]*4 | d_h*4
        WR1 = XP.tile([128, 4, 4 * O1 + NHEADS], F32, tag="WR1")
        for h in range(NHEADS):
            nc.sync.dma_start(
                WR1[:, :, O1 * h: O1 * h + O1],
                W1[h, :, :].rearrange("(c p) o -> p c o", p=128),
            )
        # w_tilde[f, h] = sum_o W1[h][f, o] * a_dst[h][o]  (host-folded)
        nc.sync.dma_start(WR1[:, :, 4 * O1: 4 * O1 + NHEADS],
                          wt1[:, :].rearrange("(c p) h -> p c h", p=128))
        WR1h = XP.tile([128, 4, 4 * O1 + NHEADS], F16, tag="WR1h")
        v_.tensor_copy(WR1h[:], WR1[:])


        # ---- s1_rep first: unblocks layer-1 attention early ----------
        s_rep1 = {}
        for h in range(NHEADS):
            wps = PSB.tile([O1, R], F32, tag="ps")
            for fc in range(4):
                t_.matmul(wps[:], WR1h[:, fc, O1 * h: O1 * h + O1], XM[:, fc, :],
                          start=(fc == 0), stop=(fc == 3))
            wsb = SP.tile([O1, R], F16, tag="whmT_sb")
            v_.tensor_copy(wsb[:], wps[:])
            sps = PSB.tile([128, R], F32, tag="ps")
            t_.matmul(sps[:], asrc_rep[(1, h)][:], wsb[:], start=True, stop=True)
            sr1 = P.tile([128, R], F16, tag=f"s_rep_{h}")
            v_.tensor_copy(sr1[:], sps[:])
            s_rep1[h] = sr1

        WhD1 = P.tile([128, NT, 4 * BLK1], F16, tag="WhD1")
        D1 = P.tile([128, NT, NHEADS], F32, tag="D1")
        WhD1v = WhD1[:, :, 0:4 * BLK1].rearrange("p t (h c) -> p t h c", c=BLK1)
        v_.memset(WhD1[:, :, O1: 4 * BLK1: BLK1], 1.0)

        # ---- layer 1: front (Wh1 projection) interleaved with ---------
        # attention superblocks so ACT starts early
        H1T = P.tile([128, 2, R], F16, tag="H1T")
        hT1 = [PSA.tile([O1 + 1, R], F32, tag=f"hT_{h}", name=f"hT_1_{h}")
               for h in range(NHEADS)]
        shiftb = None
        if EXP_SHIFT[1] != 0.0:
            shiftb = P.tile([128, 1], F32, tag="shiftb1")
            v_.memset(shiftb[:], EXP_SHIFT[1])
        XQ_ctx = tc.tile_pool(name="xq", bufs=2)
        XQ = XQ_ctx.__enter__()
        with nc.named_scope("att_l1"):
            for b in range(NSB):
                xq = XQ.tile([128, 4, SB * 128], F16, tag="xq")
                for q in range(2):
                    w = SB * 128 // 2
                    nc.sync.dma_start(
                        xq[:, :, w * q: w * (q + 1)],
                        xT[:, SB * 128 * b + w * q:
                           SB * 128 * b + w * (q + 1)].rearrange(
                            "(c p) n -> p c n", p=128))
                nc.sync.dma_start(
                    AT[:, SB * b: SB * (b + 1), :],
                    adjT[128 * SB * b: 128 * SB * (b + 1), :].rearrange(
                        "(t p) i -> p t i", p=128))
                for t in range(SB):
                    nt = SB * b + t
                    fps = PSB.tile([128, 4 * O1 + NHEADS], F32, tag="ps")
                    for fc in range(4):
                        t_.matmul(fps[:], xq[:, fc, 128 * t: 128 * t + 128],
                                  WR1h[:, fc, :], start=(fc == 0), stop=(fc == 3))
                    ceng = v_
                    ceng.tensor_copy(
                        WhD1v[:, nt, :, 0:O1],
                        fps[:, 0:4 * O1].rearrange("p (h c) -> p h c", c=O1),
                    )
                    ceng.tensor_copy(D1[:, nt, :], fps[:, 4 * O1: 4 * O1 + NHEADS])
                tail1 = None
                if b == NSB - 1:
                    def tail1(h):
                        _emit_tail(nc, SP, PSB, ones16, hT1, H1T, 1, O1, h)
                _att_sb(nc, VP, WhD1, D1, s_rep1, AT, hT1, shiftb,
                        b, blk=BLK1, tail=tail1)
        XQ_ctx.__exit__(None, None, None)
        if dbg is not None:
            nc.sync.dma_start(dbg["H1T"][:, :, :], H1T[:])

        XP_ctx.__exit__(None, None, None)
        LP_ctx = tc.tile_pool(name="late", bufs=1)
        LP = LP_ctx.__enter__()
        if _on("skip_l2"):
            nc.sync.dma_start(out[:, :], H1T[:, 0, :].rearrange("p i -> p i"))
            LP_ctx.__exit__(None, None, None)
            VP_ctx.__exit__(None, None, None)
            return
        # ---- gather phase: Wh2_mine + d2_mine -> AllGather (fp16) ----
        # Split by fc so the fc0 matmuls start as soon as heads 0/1 tails
        # finish; heads stacked in one psum bank each for wh2m and wps2.
        W2sb = LP.tile([128, 2, 4 * O2], F32, tag="W2sb")
        for h in range(NHEADS):
            nc.sync.dma_start(
                W2sb[:, :, O2 * h: O2 * h + O2],
                W2[h, :, :].rearrange("(c p) o -> p c o", p=128),
            )
        wt2sb = LP.tile([128, 2, NHEADS], F32, tag="wt2sb")
        nc.sync.dma_start(wt2sb[:],
                          wt2[:, :].rearrange("(c p) h -> p c h", p=128))
        W2h = LP.tile([128, 2, 4 * O2 + NHEADS], F16, tag="W2h")
        v_.tensor_copy(W2h[:, :, 0:4 * O2], W2sb[:])
        v_.tensor_copy(W2h[:, :, 4 * O2: 4 * O2 + NHEADS], wt2sb[:])
        _asrc_prep(2)

        Gsb = LP.tile([128, 4, GCOLS], F16, tag="Gsb")
        wh2m = PSB.tile([128, 4, NEMBED], F32, tag="ps", name="wh2m")
        d2ps = PSB.tile([128, 4, NHEADS], F32, tag="ps", name="d2ps")
        for it in range(4):
            for fc in range(2):
                t_.matmul(wh2m[:, it, :],
                          H1T[:, fc, 128 * it: 128 * it + 128],
                          W2h[:, fc, 0:4 * O2],
                          start=(fc == 0), stop=(fc == 1))
            for fc in range(2):
                t_.matmul(d2ps[:, it, :],
                          H1T[:, fc, 128 * it: 128 * it + 128],
                          W2h[:, fc, 4 * O2: 4 * O2 + NHEADS],
                          start=(fc == 0), stop=(fc == 1))
        v_.tensor_copy(Gsb[:, :, 0:NEMBED], wh2m[:])
        v_.tensor_copy(Gsb[:, :, NEMBED:GCOLS], d2ps[:])

        # ---- s2_rep (overlaps the AllGather) --------------------------
        s_rep2 = {}
        for h in range(NHEADS):
            wps = PSB.tile([O2, R], F32, tag="ps")
            for fc in range(2):
                t_.matmul(wps[:], W2h[:, fc, O2 * h: O2 * h + O2],
                          H1T[:, fc, :], start=(fc == 0), stop=(fc == 1))
            wsb = SP.tile([O2, R], F16, tag="whmT_sb")
            v_.tensor_copy(wsb[:], wps[:])
            sps = PSB.tile([128, R], F32, tag="ps")
            t_.matmul(sps[:], asrc_rep[(2, h)][:], wsb[:], start=True, stop=True)
            srt = P.tile([128, R], F16, tag=f"s_rep_{h}")
            v_.tensor_copy(srt[:], sps[:])
            s_rep2[h] = srt

        Gmine = DP.tile([R, GCOLS], F16, tag="Gmine")
        Gall = DP.tile([N, GCOLS], F16, tag="Gall", addr_space="Shared")
        nc.sync.dma_start(
            Gmine[:, :].rearrange("(t p) o -> p t o", p=128), Gsb[:])
        if _on("skip_gather"):
            nc.sync.dma_start(Gall[0:R, :], Gmine[:, :])
        else:
            nc.gpsimd.collective_compute(
                "AllGather", OP.bypass,
                replica_groups=[list(range(NCORES))],
                ins=[Gmine[:].opt()], outs=[Gall[:].opt()],
            )

        WhD2 = LP.tile([128, NT, 4 * BLK2], F16, tag="WhD2")
        D2h = LP.tile([128, NT, NHEADS], F16, tag="D2h")
        D2 = LP.tile([128, NT, NHEADS], F32, tag="D2")
        v_.memset(WhD2[:, :, O2: 4 * BLK2: BLK2], 1.0)
        for h in range(NHEADS):
            for q in range(4):
                tw = NT // 4
                nc.sync.dma_start(
                    WhD2[:, tw * q: tw * (q + 1), BLK2 * h: BLK2 * h + O2],
                    Gall[128 * tw * q: 128 * tw * (q + 1),
                         O2 * h: O2 * h + O2].rearrange("(t p) o -> p t o", p=128),
                )
        for q in range(4):
            tw = NT // 4
            nc.sync.dma_start(
                D2h[:, tw * q: tw * (q + 1), :],
                Gall[128 * tw * q: 128 * tw * (q + 1),
                     NEMBED:GCOLS].rearrange("(t p) o -> p t o", p=128),
            )
        v_.tensor_copy(D2[:], D2h[:])

        if dbg is not None:
            nc.sync.dma_start(dbg["srep2"][:, :], s_rep2[0][:])
            nc.sync.dma_start(dbg["WhD2"][:, :, :], WhD2[:])
            nc.sync.dma_start(dbg["D2"][:, :, :], D2[:])
            nc.sync.dma_start(dbg["Gsb"][:, :, :], Gsb[:])
        # ---- layer 2 attention ---------------------------------------
        H2T = LP.tile([NEMBED, R], F32, tag="H2T")
        hT2 = [PSA.tile([O2 + 1, R], F32, tag=f"hT_{h}", name=f"hT_2_{h}")
               for h in range(NHEADS)]
        with nc.named_scope("att_l2"):
            for b in range(NSB):
                tail2 = None
                if b == NSB - 1:
                    def tail2(h):
                        _emit_tail(nc, SP, PSB, ones16, hT2, H2T, 2, O2, h)
                        nc.sync.dma_start(out[O2 * h: O2 * h + O2, :],
                                          H2T[O2 * h: O2 * h + O2, :])
                _att_sb(nc, VP, WhD2, D2, s_rep2, AT, hT2, None,
                        b, blk=BLK2, tail=tail2)

        LP_ctx.__exit__(None, None, None)
        VP_ctx.__exit__(None, None, None)


def _att_sb(nc, VP, WhD, D, s_rep, AT, hT, shiftb, b, blk, tail=None):
    """Emit one attention superblock (SB j-tiles x 4 heads)."""
    v_ = nc.vector
    s_ = nc.scalar
    t_ = nc.tensor
    ndp = VARIANT.get("dve_prelu_heads", 2)
    npm = VARIANT.get("pool_mask_heads", 0)
    adj_t = AT[:, SB * b: SB * (b + 1), :]
    for h in range(NHEADS):
        v = VP.tile([128, SB, R], F16, tag="v")
        if h < ndp:
            for t in range(SB):
                jt = SB * b + t
                v_.tensor_scalar(v[:, t, :], s_rep[h][:],
                                 D[:, jt, h: h + 1], None, op0=OP.add)
        elif _on("no_bias_prelu"):
            for t in range(SB):
                jt = SB * b + t
                v_.tensor_scalar(v[:, t, :], s_rep[h][:],
                                 D[:, jt, h: h + 1], None, op0=OP.add)
        else:
            # fused add+prelu on ACT: v = Prelu(s_rep + d_j)
            for t in range(SB):
                jt = SB * b + t
                s_.activation(v[:, t, :], s_rep[h][:], AF.Prelu,
                              bias=D[:, jt, h: h + 1], scale=1.0, alpha=ALPHA)
        hs = SB // 2
        for ph in range(2):
            vv = v[:, hs * ph: hs * (ph + 1), :]
            aa = adj_t[:, hs * ph: hs * (ph + 1), :]
            if h < ndp:
                # prelu on DVE (+pool): max(z, 0.2z)
                q = VP.tile([128, hs, R], F16, tag="q")
                qeng = nc.gpsimd if _on("q_pool") else v_
                qeng.tensor_scalar(q[:], vv, 0.2, None, op0=OP.mult)
                v_.tensor_tensor(vv, vv, q[:], op=OP.max)
            elif _on("no_bias_prelu"):
                s_.activation(vv, vv, AF.Prelu,
                              bias=0.0, scale=1.0, alpha=ALPHA)
            s_.activation(vv, vv, AF.Exp,
                          bias=(shiftb[:] if shiftb is not None else 0.0))
            meng = nc.gpsimd if h >= NHEADS - npm else v_
            meng.tensor_tensor(vv, vv, aa, op=OP.mult)
        for t in range(SB):
            jt = SB * b + t
            t_.matmul(hT[h][:], WhD[:, jt, blk * h: blk * h + blk],
                      v[:, t, :], start=(jt == 0), stop=(jt == NT - 1))
        if tail is not None:
            tail(h)


VARIANT = {}


def _on(flag):
    return VARIANT.get(flag, False)


def _emit_tail(nc, SP, PSB, ones16, hT, Hout, layer, Fo, h):
    v_ = nc.vector
    s_ = nc.scalar
    t_ = nc.tensor
    r1 = SP.tile([1, R], F16, tag="recip", name=f"r1_{layer}_{h}")
    with nc.allow_low_precision(reason="fp16 recip: 5e-4 rel, tol is 2e-2"):
        v_.reciprocal(r1[:], hT[h][Fo: Fo + 1, :])
    rps = PSB.tile([128, R], F32, tag="ps", name=f"rps_{layer}_{h}")
    t_.matmul(rps[:], ones16[:], r1[:], start=True, stop=True)
    teng = nc.gpsimd if _on("tail_pool") else v_
    rrep = SP.tile([128, R], F16, tag="rrep", name=f"rrep_{layer}_{h}")
    if _on("rrep_act"):
        s_.activation(rrep[:], rps[:], AF.Copy)
    else:
        teng.tensor_copy(rrep[:], rps[:])
    hn = SP.tile([Fo, R], F16, tag="hn", name=f"hn_{layer}_{h}")
    teng.tensor_tensor(hn[:], hT[h][0:Fo, :], rrep[0:Fo, :], op=OP.mult)
    # ELU(x) = max(x,0) - 1 + exp(min(x,0))
    m = SP.tile([Fo, R], F16, tag="elu_m", name=f"m_{layer}_{h}")
    v_.tensor_scalar(m[:], hn[:], 0.0, None, op0=OP.min)
    s_.activation(m[:], m[:], AF.Exp)
    rl = SP.tile([Fo, R], F16, tag="elu_rl", name=f"rl_{layer}_{h}")
    v_.tensor_scalar(rl[:], hn[:], 0.0, -1.0, op0=OP.max, op1=OP.add)
    if layer == 1:
        dst = Hout[64 * (h % 2): 64 * (h % 2) + 64, h // 2, :]
    else:
        dst = Hout[O2 * h: O2 * h + O2, :]
    teng.tensor_tensor(dst, m[:], rl[:], op=OP.add)


_NC_CACHE = {}


def _get_nc():
    if "nc" not in _NC_CACHE:
        _NC_CACHE["nc"] = _build()
    return _NC_CACHE["nc"]


def _in_maps_for(inputs):
    x = np.ascontiguousarray(np.asarray(inputs["x"], dtype=np.float32))
    adj = np.asarray(inputs["adj"], dtype=np.float32)
    xT = np.ascontiguousarray(x.T.astype(np.float16))
    a1f = np.ascontiguousarray(np.asarray(inputs["a1"], np.float32).reshape(1, -1))
    a2f = np.ascontiguousarray(np.asarray(inputs["a2"], np.float32).reshape(1, -1))
    W1c = np.ascontiguousarray(np.asarray(inputs["W1"], np.float32))
    a1c = np.asarray(inputs["a1"], np.float32)
    wt1 = np.ascontiguousarray(
        np.einsum("hfo,ho->fh", W1c, a1c[:, O1:]).astype(np.float32))
    W2c = np.ascontiguousarray(np.asarray(inputs["W2"], np.float32))
    a2c = np.asarray(inputs["a2"], np.float32)
    wt2 = np.ascontiguousarray(
        np.einsum("hfo,ho->fh", W2c, a2c[:, O2:]).astype(np.float32))

    in_maps = []
    for c in range(NCORES):
        rows = slice(R * c, R * (c + 1))
        in_maps.append({
            "xT": xT,
            "xmT": np.ascontiguousarray(x[rows, :].T.astype(np.float16)),
            "adjT": np.ascontiguousarray(adj[rows, :].T.astype(np.float16)),
            "W1": W1c, "wt1": wt1, "wt2": wt2, "a1f": a1f,
            "W2": W2c, "a2f": a2f,
        })
    return in_maps


def kernel(x, adj, W1, a1, W2, a2):
    nc = _get_nc()
    in_maps = _in_maps_for(dict(x=x, adj=adj, W1=W1, a1=a1, W2=W2, a2=a2))
    res = run_bass_kernel_spmd(nc, in_maps, core_ids=list(range(NCORES)))
    return np.concatenate(
        [np.asarray(res.results[c]["h2T"]).T for c in range(NCORES)], axis=0)



# revision 26
# speedup vs baseline: 1.6650x; 1.6650x over previous
"""Dense 2-layer GAT (4 heads) on 8 Trainium2 NeuronCores.

Distribution: 1D row-parallel over destination nodes. Core c owns rows
R_c = [512c, 512c+512). Each core computes its rows of both GAT layers;
a mid-kernel AllGather exchanges the layer-2 projections [Wh2 | d2]
(fp16, 1.1 MB) instead of h1 (the only cross-core dependency).

On-device layout: attention is built TRANSPOSED, att[j, i] (source node j
on partitions, my rows i on free), so
  - hT[o, i] = sum_j Wh[j, o] * att[j, i] needs no transpose of att,
  - softmax denominators come free as an extra ones-column in lhsT,
  - the layer output hT is exactly the lhsT the next layer's projection
    needs.
x is pre-transposed and cast to fp16 on host (layout/dtype-only prep);
adj is pre-transposed, column-sliced per core, and cast to fp16 ({0,1}
exact). Output is produced as h2T [128, 512] per core and un-transposed
on host.

Everything on the N^2 path is fp16: matmuls run at 1 cycle/row (fp32
is 4), DVE tensor_scalar hits 4x mode and tensor_tensor 2x. The
adjacency tile (4 MB fp16) is loaded ONCE and stays resident in SBUF
for both layers. Layer-1 exp uses a global -2 bias (softmax-exact
shift) so exp(z) stays well inside fp16 range. a_dst is folded into
the weights on host (wt1/wt2), so per-node d values fall out of the
projection matmuls directly.

The logits pipeline is balanced across ScalarE and DVE per head
(dve_prelu_heads=2): heads 0-1 build z on DVE and compute
prelu = max(z, 0.2z) with a DVE mult+max; heads 2-3 fuse the z-build
into ScalarE's Prelu via its per-partition bias port. Exp always runs
on ScalarE, the {0,1} mask multiply on DVE, and 8 accumulating fp16
matmuls per superblock produce hT (+denominator row) in psum. The
layer-1 projection front is interleaved superblock-by-superblock with
attention so ScalarE starts ~15us into the kernel, tails are emitted
inline per head, and the layer-2 gather phase (Wh2|d2 via one fused
matmul block) feeds a fp16 AllGather overlapped with s_rep2 compute.
"""
import sys

if "/opt/trn_rl_repo" not in sys.path:
    sys.path.insert(0, "/opt/trn_rl_repo")

import numpy as np

import concourse.bacc as bacc
import concourse.mybir as mybir
import concourse.tile as tile
from concourse.bass_utils import run_bass_kernel_spmd

F32 = mybir.dt.float32
F16 = mybir.dt.float16
I16 = mybir.dt.int16
AF = mybir.ActivationFunctionType
OP = mybir.AluOpType

N = 4096
NFEAT = 512
NHID = 256
NEMBED = 128
NHEADS = 4
O1 = 64
O2 = 32
NCORES = 8
R = N // NCORES          # 512 rows per core
ALPHA = 0.2
NT = N // 128            # 32 j-tiles
SB = 8                   # j-tiles per superblock
NSB = NT // SB           # 4 superblocks
BLK1 = O1 + 1            # 65: [Wh_h | ones]
BLK2 = O2 + 1            # 33
GCOLS = NEMBED + NHEADS  # 132: [Wh2 (128) | d2 (4)]
EXP_SHIFT = {1: -2.0, 2: -2.0}  # softmax-invariant bias on the exp
# Schraudolph exp-via-bitcast: u = bitcast_f16(int16(w' + ATB)) where
# w' = SCHA*prelu(z) (SCHA folded into the host-side a_src / a_dst-in-W
# prep, so z' = SCHA*z and ACT Prelu commutes with the scale) and ATB is
# the adjacency tile remapped to {SCHB_l (edge), -64000 (no edge)}. The
# int16 convert truncates and SATURATES: masked entries -> -32768 =
# 0x8000 = f16 -0.0, i.e. a zero attention weight - mask, bias and exp
# all ride one tensor_tensor add. SCHB centers the log-linear sawtooth
# (-45) and folds the per-layer softmax-invariant EXP_SHIFT.
SCHA = 1024.0 / float(np.log(2.0))
SCHB = {l: 15360.0 - 45.0 + SCHA * EXP_SHIFT[l] for l in (1, 2)}
ATB_MASKED = -64000.0


def _build(debug=False, repeat=1):
    nc = bacc.Bacc("TRN2", target_bir_lowering=False, debug=False,
                   num_devices=NCORES)

    xmT = nc.dram_tensor("xmT", [NFEAT, R], F16, kind="ExternalInput").ap()
    adjT = nc.dram_tensor("adjT", [N, R], F16, kind="ExternalInput").ap()
    w1h = nc.dram_tensor("w1h", [128, 4, 4 * O1 + NHEADS], F16,
                         kind="ExternalInput").ap()
    w2h = nc.dram_tensor("w2h", [128, 2, 4 * O2 + NHEADS], F16,
                         kind="ExternalInput").ap()
    a1f = nc.dram_tensor("a1f", [1, 2 * O1 * NHEADS], F32, kind="ExternalInput").ap()
    a2f = nc.dram_tensor("a2f", [1, 2 * O2 * NHEADS], F32, kind="ExternalInput").ap()
    out = nc.dram_tensor("h2T", [NEMBED, R], F32, kind="ExternalOutput").ap()
    dbg = None
    if debug:
        dbg = {
            "H1T": nc.dram_tensor("d_H1T", [128, 2, R], F16,
                                  kind="ExternalOutput").ap(),
            "srep2": nc.dram_tensor("d_srep2", [128, R], F16,
                                    kind="ExternalOutput").ap(),
            "WhD2": nc.dram_tensor("d_WhD2", [128, NT, 4 * BLK2], F16,
                                   kind="ExternalOutput").ap(),
            "D2": nc.dram_tensor("d_D2", [128, NT, NHEADS], F32,
                                 kind="ExternalOutput").ap(),
            "Gsb": nc.dram_tensor("d_Gsb", [128, 4, GCOLS], F16,
                                  kind="ExternalOutput").ap(),
        }

    with tile.TileContext(nc) as tc:
        for _rep in range(repeat):
            _emit(tc, nc, xmT, adjT, w1h, w2h, a1f, a2f, out, dbg)
    nc.compile()
    return nc


def _emit(tc, nc, xmT, adjT, w1h, w2h, a1f, a2f, out, dbg):
    v_ = nc.vector
    s_ = nc.scalar
    t_ = nc.tensor

    with (
        tc.tile_pool(name="persist", bufs=1) as P,
        tc.tile_pool(name="small", bufs=VARIANT.get("sp_bufs", 2)) as SP,
        tc.tile_pool(name="psA", bufs=1, space="PSUM") as PSA,
        tc.tile_pool(name="psB", bufs=VARIANT.get("psb_bufs", 4), space="PSUM") as PSB,
        tc.tile_pool(name="dram", bufs=1, space="DRAM") as DP,
    ):
        XM = P.tile([128, 4, R], F16, tag="XM")
        nc.sync.dma_start(XM[:], xmT[:, :].rearrange("(c p) n -> p c n", p=128))
        ones32 = P.tile([1, 128], F32, tag="ones32")
        v_.memset(ones32[:], 1.0)
        ones16 = P.tile([1, 128], F16, tag="ones16")
        v_.memset(ones16[:], 1.0)

        # ---- resident adjacency (ATB encoding): streamed in during the
        # front + L1 AllGather, reused (remapped) by layer 2
        AT = P.tile([128, NT, R], F16, tag="AT")

        def _load_adj(b):
            nc.gpsimd.dma_start(
                AT[:, SB * b: SB * (b + 1), :],
                adjT[128 * SB * b: 128 * SB * (b + 1), :].rearrange(
                    "(t p) i -> p t i", p=128))
        _load_adj(0)

        # ---- per-head a_src prep for both layers ----------------------
        # One contiguous DMA per layer; broadcasts built via K=1 matmuls.
        asrc_rep = {}   # (l, h) -> [Fo, 128] fp16 a_src broadcast along free
        afsb = {}
        for l, af, Fo in ((1, a1f, O1), (2, a2f, O2)):
            asb = P.tile([1, 2 * Fo * NHEADS], F32, tag=f"afsb{l}")
            nc.sync.dma_start(asb[:], af[:, :])
            afsb[l] = (asb, Fo)
        def _asrc_prep(l):
            asb, Fo = afsb[l]
            for h in range(NHEADS):
                aps = PSB.tile([Fo, 128], F32, tag="ps")
                t_.matmul(aps[:], asb[0:1, 2 * Fo * h: 2 * Fo * h + Fo],
                          ones32[:], start=True, stop=True)
                rep = P.tile([Fo, 128], F16, tag=f"asrc_rep{l}_{h}")
                v_.tensor_copy(rep[:], aps[:])
                asrc_rep[(l, h)] = rep
        _asrc_prep(1)

        VP_ctx = tc.tile_pool(name="vwork", bufs=VARIANT.get("v_bufs", 4))
        VP = VP_ctx.__enter__()
        XP_ctx = tc.tile_pool(name="xload", bufs=1)
        XP = XP_ctx.__enter__()
        # ---- layer-1 front: [W1 all heads | w_tilde] prearranged f16 ---
        WR1h = XP.tile([128, 4, 4 * O1 + NHEADS], F16, tag="WR1h")
        nc.sync.dma_start(WR1h[:], w1h[:, :, :])


        WhD1 = P.tile([128, NT, 4 * BLK1], F16, tag="WhD1")
        D1 = P.tile([128, NT, NHEADS], F32, tag="D1")
        v_.memset(WhD1[:, :, O1: 4 * BLK1: BLK1], 1.0)

        # ---- project my 512 rows -> [Wh1 | d1] -> AllGather ASAP ------
        G1C = 4 * O1 + NHEADS
        Gsb1 = XP.tile([128, 4, G1C], F16, tag="Gsb1")
        Gmine1 = DP.tile([R, G1C], F16, tag="Gmine1")
        Gall1 = DP.tile([N, G1C], F16, tag="Gall1", addr_space="Shared")
        for it in range(4):
            g1ps = PSB.tile([128, G1C], F32, tag="ps", name=f"g1ps_{it}")
            for fc in range(4):
                t_.matmul(g1ps[:], XM[:, fc, 128 * it: 128 * it + 128],
                          WR1h[:, fc, :], start=(fc == 0), stop=(fc == 3))
            v_.tensor_copy(Gsb1[:, it, :], g1ps[:])
            nc.sync.dma_start(
                Gmine1[128 * it: 128 * it + 128, :].rearrange(
                    "(t p) o -> p t o", p=128),
                Gsb1[:, it: it + 1, :])
        if _on("skip_gather"):
            nc.sync.dma_start(Gall1[0:R, :], Gmine1[:, :])
        else:
            nc.gpsimd.collective_compute(
                "AllGather", OP.bypass,
                replica_groups=[list(range(NCORES))],
                ins=[Gmine1[:].opt()], outs=[Gall1[:].opt()],
            )

        # ---- s1_rep: local, fills the AllGather latency ---------------
        s_rep1 = {}
        for h in range(NHEADS):
            wps = PSB.tile([O1, R], F32, tag="ps")
            for fc in range(4):
                t_.matmul(wps[:], WR1h[:, fc, O1 * h: O1 * h + O1], XM[:, fc, :],
                          start=(fc == 0), stop=(fc == 3))
            wsb = SP.tile([O1, R], F16, tag="whmT_sb")
            ceng = s_ if _on("front_act_copies") else v_
            (ceng.copy if ceng is s_ else ceng.tensor_copy)(wsb[:], wps[:])
            sps = PSB.tile([128, R], F32, tag="ps")
            t_.matmul(sps[:], asrc_rep[(1, h)][:], wsb[:], start=True, stop=True)
            sr1 = P.tile([128, R], F16, tag=f"s_rep_{h}")
            (ceng.copy if ceng is s_ else ceng.tensor_copy)(sr1[:], sps[:])
            s_rep1[h] = sr1

        # ---- load back [Wh1 | d1] per quarter (batched 4D-AP DMAs) ----
        D1h = XP.tile([128, NT, NHEADS], F16, tag="D1h")
        tw = NT // 4
        for q in range(4):
            for h in range(NHEADS):
                nc.sync.dma_start(
                    WhD1[:, tw * q: tw * (q + 1), BLK1 * h: BLK1 * h + O1],
                    Gall1[128 * tw * q: 128 * tw * (q + 1),
                          O1 * h: O1 * h + O1].rearrange(
                              "(t p) o -> p t o", p=128),
                )
            nc.sync.dma_start(
                D1h[:, tw * q: tw * (q + 1), :],
                Gall1[128 * tw * q: 128 * tw * (q + 1),
                      4 * O1: G1C].rearrange("(t p) o -> p t o", p=128),
            )
            v_.tensor_copy(D1[:, tw * q: tw * (q + 1), :],
                           D1h[:, tw * q: tw * (q + 1), :])
        for b in range(1, NSB):
            _load_adj(b)

        # ---- layer 1: front (Wh1 projection) interleaved with ---------
        # attention superblocks so ACT starts early
        H1T = P.tile([128, 2, R], F16, tag="H1T")
        hT1 = [PSA.tile([O1 + 1, R], F32, tag=f"hT_{h}", name=f"hT_1_{h}")
               for h in range(NHEADS)]
        # ---- L2 weight prep, hoisted to overlap L1 attention ----------
        W2h = P.tile([128, 2, 4 * O2 + NHEADS], F16, tag="W2h")
        nc.sync.dma_start(W2h[:], w2h[:, :, :])
        _asrc_prep(2)
        with nc.named_scope("att_l1"):
            for b in range(NSB):
                tail1 = None
                if b == NSB - 1:
                    def tail1(h):
                        _emit_tail(nc, SP, PSB, ones16, hT1, H1T, 1, O1, h)
                _att_sb(nc, VP, WhD1, D1, s_rep1, AT, hT1,
                        b, blk=BLK1, layer=1, tail=tail1)
        if dbg is not None:
            nc.sync.dma_start(dbg["H1T"][:, :, :], H1T[:])

        XP_ctx.__exit__(None, None, None)
        LP_ctx = tc.tile_pool(name="late", bufs=1)
        LP = LP_ctx.__enter__()
        if _on("skip_l2"):
            nc.sync.dma_start(out[:, :], H1T[:, 0, :].rearrange("p i -> p i"))
            LP_ctx.__exit__(None, None, None)
            VP_ctx.__exit__(None, None, None)
            return
        # ---- gather phase: Wh2_mine + d2_mine -> AllGather (fp16) ----
        # Split by fc so the fc0 matmuls start as soon as heads 0/1 tails
        # finish; heads stacked in one psum bank each for wh2m and wps2.
        Gsb = LP.tile([128, 4, GCOLS], F16, tag="Gsb")
        wh2m = PSB.tile([128, 4, NEMBED], F32, tag="ps", name="wh2m")
        d2ps = PSB.tile([128, 4, NHEADS], F32, tag="ps", name="d2ps")
        for it in range(4):
            for fc in range(2):
                t_.matmul(wh2m[:, it, :],
                          H1T[:, fc, 128 * it: 128 * it + 128],
                          W2h[:, fc, 0:4 * O2],
                          start=(fc == 0), stop=(fc == 1))
            for fc in range(2):
                t_.matmul(d2ps[:, it, :],
                          H1T[:, fc, 128 * it: 128 * it + 128],
                          W2h[:, fc, 4 * O2: 4 * O2 + NHEADS],
                          start=(fc == 0), stop=(fc == 1))
        v_.tensor_copy(Gsb[:, :, 0:NEMBED], wh2m[:])
        v_.tensor_copy(Gsb[:, :, NEMBED:GCOLS], d2ps[:])

        # ---- s2_rep (overlaps the AllGather) --------------------------
        s_rep2 = {}
        for h in range(NHEADS):
            wps = PSB.tile([O2, R], F32, tag="ps")
            for fc in range(2):
                t_.matmul(wps[:], W2h[:, fc, O2 * h: O2 * h + O2],
                          H1T[:, fc, :], start=(fc == 0), stop=(fc == 1))
            wsb = SP.tile([O2, R], F16, tag="whmT_sb")
            v_.tensor_copy(wsb[:], wps[:])
            sps = PSB.tile([128, R], F32, tag="ps")
            t_.matmul(sps[:], asrc_rep[(2, h)][:], wsb[:], start=True, stop=True)
            srt = P.tile([128, R], F16, tag=f"s_rep_{h}")
            v_.tensor_copy(srt[:], sps[:])
            s_rep2[h] = srt

        Gmine = DP.tile([R, GCOLS], F16, tag="Gmine")
        Gall = DP.tile([N, GCOLS], F16, tag="Gall", addr_space="Shared")
        nc.sync.dma_start(
            Gmine[:, :].rearrange("(t p) o -> p t o", p=128), Gsb[:])
        if _on("skip_gather"):
            nc.sync.dma_start(Gall[0:R, :], Gmine[:, :])
        else:
            nc.gpsimd.collective_compute(
                "AllGather", OP.bypass,
                replica_groups=[list(range(NCORES))],
                ins=[Gmine[:].opt()], outs=[Gall[:].opt()],
            )

        WhD2 = LP.tile([128, NT, 4 * BLK2], F16, tag="WhD2")
        D2h = LP.tile([128, NT, NHEADS], F16, tag="D2h")
        D2 = LP.tile([128, NT, NHEADS], F32, tag="D2")
        v_.memset(WhD2[:, :, O2: 4 * BLK2: BLK2], 1.0)
        for q in range(4):
            tw = NT // 4
            for h in range(NHEADS):
                nc.sync.dma_start(
                    WhD2[:, tw * q: tw * (q + 1), BLK2 * h: BLK2 * h + O2],
                    Gall[128 * tw * q: 128 * tw * (q + 1),
                         O2 * h: O2 * h + O2].rearrange(
                             "(t p) o -> p t o", p=128),
                )
        for q in range(4):
            tw = NT // 4
            nc.sync.dma_start(
                D2h[:, tw * q: tw * (q + 1), :],
                Gall[128 * tw * q: 128 * tw * (q + 1),
                     NEMBED:GCOLS].rearrange("(t p) o -> p t o", p=128),
            )
            v_.tensor_copy(D2[:, tw * q: tw * (q + 1), :],
                           D2h[:, tw * q: tw * (q + 1), :])

        if dbg is not None:
            nc.sync.dma_start(dbg["srep2"][:, :], s_rep2[0][:])
            nc.sync.dma_start(dbg["WhD2"][:, :, :], WhD2[:])
            nc.sync.dma_start(dbg["D2"][:, :, :], D2[:])
            nc.sync.dma_start(dbg["Gsb"][:, :, :], Gsb[:])
        # ---- layer 2 attention ---------------------------------------
        H2T = LP.tile([NEMBED, R], F32, tag="H2T")
        hT2 = [PSA.tile([O2 + 1, R], F32, tag=f"hT_{h}", name=f"hT_2_{h}")
               for h in range(NHEADS)]
        with nc.named_scope("att_l2"):
            for b in range(NSB):
                tail2 = None
                if b == NSB - 1:
                    def tail2(h):
                        _emit_tail(nc, SP, PSB, ones16, hT2, H2T, 2, O2, h)
                        nc.sync.dma_start(out[O2 * h: O2 * h + O2, :],
                                          H2T[O2 * h: O2 * h + O2, :])
                _att_sb(nc, VP, WhD2, D2, s_rep2, AT, hT2,
                        b, blk=BLK2, layer=2, tail=tail2)

        LP_ctx.__exit__(None, None, None)
        VP_ctx.__exit__(None, None, None)


def _att_sb(nc, VP, WhD, D, s_rep, AT, hT, b, blk, layer=1, tail=None):
    """Emit one attention superblock (SB j-tiles x 4 heads).

    Per head: z-build on DVE (ts, 4x), Prelu on ACT in-place, then the exp
    is ABSORBED into a dtype conversion (Schraudolph): bits = int16(
    SCHA*w + SCHB) via one 4x tensor_scalar, mask applied as an integer
    multiply against the {0,1} adjacency, and the aggregation matmul reads
    the masked bits tile bitcast as f16 — the weights are e^(w+shift) with
    ~2-3%/value sawtooth error, well inside the 2e-2 tolerance.
    """
    v_ = nc.vector
    s_ = nc.scalar
    t_ = nc.tensor
    g_ = nc.gpsimd
    npm = VARIANT.get("pool_phs", 2)
    adj_t = AT[:, SB * b: SB * (b + 1), :]
    for h in range(NHEADS):
        v = VP.tile([128, SB, R], F16, tag="v")
        vb = VP.tile([128, SB, R], I16, tag="vb")
        nph = VARIANT.get("nph", 2)
        hs = SB // nph
        for ph in range(nph):
            vv = v[:, hs * ph: hs * (ph + 1), :]
            ib = vb[:, hs * ph: hs * (ph + 1), :]
            aa = adj_t[:, hs * ph: hs * (ph + 1), :]
            for t in range(hs * ph, hs * (ph + 1)):
                jt = SB * b + t
                v_.tensor_scalar(v[:, t, :], s_rep[h][:],
                                 D[:, jt, h: h + 1], None, op0=OP.add)
            s_.activation(vv, vv, AF.Prelu, bias=0.0, scale=1.0, alpha=ALPHA)
            if 2 * h + ph < npm:
                # Pool legality: integer tt needs matching dtypes, so the
                # saturating f16 add stays f16 on Pool and DVE converts.
                g_.tensor_tensor(vv, vv, aa, op=OP.add)
                v_.tensor_scalar(ib, vv, 0.0, None, op0=OP.add)
            else:
                v_.tensor_tensor(ib, vv, aa, op=OP.add)
            for t in range(hs * ph, hs * (ph + 1)):
                jt = SB * b + t
                t_.matmul(hT[h][:], WhD[:, jt, blk * h: blk * h + blk],
                          vb[:, t, :].bitcast(F16),
                          start=(jt == 0), stop=(jt == NT - 1))
        if tail is not None:
            tail(h)


VARIANT = {}


def _on(flag):
    return VARIANT.get(flag, False)


def _emit_tail(nc, SP, PSB, ones16, hT, Hout, layer, Fo, h):
    v_ = nc.vector
    s_ = nc.scalar
    t_ = nc.tensor
    r1 = SP.tile([1, R], F16, tag="recip", name=f"r1_{layer}_{h}")
    with nc.allow_low_precision(reason="fp16 recip: 5e-4 rel, tol is 2e-2"):
        v_.reciprocal(r1[:], hT[h][Fo: Fo + 1, :])
    rps = PSB.tile([128, R], F32, tag="ps", name=f"rps_{layer}_{h}")
    t_.matmul(rps[:], ones16[:], r1[:], start=True, stop=True)
    teng = nc.gpsimd if _on("tail_pool") else v_
    rrep = SP.tile([128, R], F16, tag="rrep", name=f"rrep_{layer}_{h}")
    if _on("rrep_act"):
        s_.activation(rrep[:], rps[:], AF.Copy)
    else:
        teng.tensor_copy(rrep[:], rps[:])
    hn = SP.tile([Fo, R], F16, tag="hn", name=f"hn_{layer}_{h}")
    teng.tensor_tensor(hn[:], hT[h][0:Fo, :], rrep[0:Fo, :], op=OP.mult)
    # ELU(x) = max(x,0) - 1 + exp(min(x,0))
    m = SP.tile([Fo, R], F16, tag="elu_m", name=f"m_{layer}_{h}")
    v_.tensor_scalar(m[:], hn[:], 0.0, None, op0=OP.min)
    s_.activation(m[:], m[:], AF.Exp)
    rl = SP.tile([Fo, R], F16, tag="elu_rl", name=f"rl_{layer}_{h}")
    v_.tensor_scalar(rl[:], hn[:], 0.0, -1.0, op0=OP.max, op1=OP.add)
    if layer == 1:
        dst = Hout[64 * (h % 2): 64 * (h % 2) + 64, h // 2, :]
    else:
        dst = Hout[O2 * h: O2 * h + O2, :]
    teng.tensor_tensor(dst, m[:], rl[:], op=OP.add)


_NC_CACHE = {}


def _get_nc():
    if "nc" not in _NC_CACHE:
        _NC_CACHE["nc"] = _build()
    return _NC_CACHE["nc"]


def _in_maps_for(inputs):
    x = np.ascontiguousarray(np.asarray(inputs["x"], dtype=np.float32))
    adj = np.asarray(inputs["adj"], dtype=np.float32)
    # SCHA pre-scales the logits: s and d both come out of [a_src | wt]
    # projections, so the scale rides the host-side a/wt prep for free.
    a1f = np.ascontiguousarray(
        SCHA * np.asarray(inputs["a1"], np.float32).reshape(1, -1))
    a2f = np.ascontiguousarray(
        SCHA * np.asarray(inputs["a2"], np.float32).reshape(1, -1))
    W1c = np.asarray(inputs["W1"], np.float32)
    a1c = np.asarray(inputs["a1"], np.float32)
    wt1 = SCHA * np.einsum("hfo,ho->fh", W1c, a1c[:, O1:])
    W2c = np.asarray(inputs["W2"], np.float32)
    a2c = np.asarray(inputs["a2"], np.float32)
    wt2 = SCHA * np.einsum("hfo,ho->fh", W2c, a2c[:, O2:])
    # device-layout weight blocks [128, fc, 4*Fo + H] fp16
    w1flat = np.concatenate(
        [np.concatenate(list(W1c), axis=1), wt1], axis=1)  # [512, 260]
    w1h = np.ascontiguousarray(
        w1flat.reshape(4, 128, 4 * O1 + NHEADS).transpose(1, 0, 2)
        .astype(np.float16))
    w2flat = np.concatenate(
        [np.concatenate(list(W2c), axis=1), wt2], axis=1)  # [256, 132]
    w2h = np.ascontiguousarray(
        w2flat.reshape(2, 128, 4 * O2 + NHEADS).transpose(1, 0, 2)
        .astype(np.float16))
    # adjacency -> ATB encoding: SCHB[1] on edges, saturating bias off them
    atb = np.where(adj > 0, np.float32(SCHB[1]),
                   np.float32(ATB_MASKED)).astype(np.float16)

    in_maps = []
    for c in range(NCORES):
        rows = slice(R * c, R * (c + 1))
        in_maps.append({
            "xmT": np.ascontiguousarray(x[rows, :].T.astype(np.float16)),
            "adjT": np.ascontiguousarray(atb[rows, :].T),
            "w1h": w1h, "w2h": w2h, "a1f": a1f, "a2f": a2f,
        })
    return in_maps


def kernel(x, adj, W1, a1, W2, a2):
    nc = _get_nc()
    in_maps = _in_maps_for(dict(x=x, adj=adj, W1=W1, a1=a1, W2=W2, a2=a2))
    res = run_bass_kernel_spmd(nc, in_maps, core_ids=list(range(NCORES)))
    return np.concatenate(
        [np.asarray(res.results[c]["h2T"]).T for c in range(NCORES)], axis=0)

